# revision 7
# baseline (speedup 1.0000x reference)
"""Trainium2 Bass kernel for nn_CBERT: two-tower cross-attention BERT with
top-2-of-8 MoE FFNs, run SPMD across 8 NeuronCores.

Sharding: expert-parallel MoE (core k owns expert k, dense masked compute,
AllReduce combine); attention/LN/gate replicated on all cores; weights for
the non-expert parts are broadcast.

Layout: activations live on-chip as x^T [128 (feature part), 6 (chunk), T
(tokens)] in float32r; attention q/k/v/probs and the big FFN matmuls run in
bf16; LayerNorm statistics, gating and residuals in fp32/f32r.
"""

import sys

for _p in ("/opt/trn_rl_repo",):
    if _p not in sys.path:
        sys.path.insert(0, _p)

import numpy as np
import ml_dtypes

import concourse.bass as bass
import concourse.mybir as mybir
import concourse.tile as tile
from concourse import bacc
from concourse.masks import make_identity

# ---- model dims (hardcoded from the problem spec) ----
N_CORES = 8
B, LT, LC = 2, 128, 256
D, I, H, E, NL = 768, 3072, 12, 8, 2
HD = 64
DC = D // 128          # 6 feature chunks
ICN = I // 128         # 24 intermediate chunks
T_T, T_C = B * LT, B * LC   # 256, 512
EPS = 1e-12
NEG = -1e4

f32 = mybir.dt.float32
f32r = mybir.dt.float32r
bf16 = mybir.dt.bfloat16
AF = mybir.ActivationFunctionType
ALU = mybir.AluOpType
BF16 = ml_dtypes.bfloat16

INSTANCES = [("t0", "t", 0), ("c0", "c", 0), ("t1", "t", 1), ("c1", "c", 1)]
TOWER_T = {"t": T_T, "c": T_C}
TOWER_L = {"t": LT, "c": LC}

_uid = [0]


def _nm(p):
    _uid[0] += 1
    return f"{p}_{_uid[0]}"


def ts(i, sz=128):
    return slice(i * sz, (i + 1) * sz)


# ============================================================================
# device program
# ============================================================================

def build_nc(flags):
    nc = bacc.Bacc("TRN2", target_bir_lowering=False, num_devices=N_CORES)

    # ---- DRAM I/O ----
    dram = {}

    def din(name, shape, dt):
        dram[name] = nc.dram_tensor(name, list(shape), dt, kind="ExternalInput")
        return dram[name]

    for key, tw, i in INSTANCES:
        din(f"wq_{key}", (128, DC, D), bf16)
        din(f"wk_{key}", (128, DC, D), bf16)
        din(f"wv_{key}", (128, DC, D), bf16)
        din(f"wo_{key}", (128, DC, D), bf16)
        din(f"bq_{key}", (128, DC), f32)
        din(f"bk_{key}", (128, DC), f32)
        din(f"bo_{key}", (128, DC), f32)
        if flags[f"need_bv_{key}"]:
            din(f"bvrow_{key}", (1, D), f32r)
        din(f"gw_{key}", (128, DC, E), f32r)
        if flags[f"need_gb_{key}"]:
            din(f"gbrow_{key}", (1, E), f32r)
        din(f"w1_{key}", (ICN, 128, DC * 128), bf16)
        din(f"b1_{key}", (128, ICN), f32)
        din(f"w2_{key}", (DC, ICN, 128, 128), bf16)
        din(f"b2_{key}", (128, DC), f32)
        for ln_i in (1, 2):
            din(f"lng{ln_i}_{key}", (1, D), f32r)
            din(f"lnb{ln_i}_{key}", (1, D), f32r)
    din("xt_t", (128, DC, T_T), f32r)
    din("xt_c", (128, DC, T_C), f32r)
    din("esel", (E, 128), f32r)
    if flags["need_mask_t"]:
        din("mextrow_t", (1, T_T), f32r)
    if flags["need_mask_c"]:
        din("mextrow_c", (1, T_C), f32r)
    din("w1ca", (12, 128, D), f32r)
    din("w1cb", (4, D), f32r)
    din("b1c", (128, DC), f32)
    din("w2c", (128, DC, E), f32r)
    if flags["need_b2c"]:
        din("b2crow", (1, 8), f32r)
    din("efT", (4, 2), f32r)
    din("chain", (1, 1), f32)

    out_d = nc.dram_tensor("out", [2, 8], f32, kind="ExternalOutput")
    chain_d = nc.dram_tensor("chain_out", [1, 1], f32, kind="ExternalOutput")
    taps = {}
    if flags["tap"]:
        for key, tw, i in INSTANCES:
            taps[key] = {
                "x": nc.dram_tensor(f"tap_x_{key}", [128, DC, TOWER_T[tw]], f32,
                                    kind="ExternalOutput"),
                "y": nc.dram_tensor(f"tap_y_{key}", [128, DC, TOWER_T[tw]], f32,
                                    kind="ExternalOutput"),
                "x2": nc.dram_tensor(f"tap_x2_{key}", [128, DC, TOWER_T[tw]], f32,
                                     kind="ExternalOutput"),
                "ms": nc.dram_tensor(f"tap_ms_{key}", [128, DC, TOWER_T[tw]], f32,
                                     kind="ExternalOutput"),
                "w8": nc.dram_tensor(f"tap_w8_{key}", [E, TOWER_T[tw]], f32,
                                     kind="ExternalOutput"),
            }

    with tile.TileContext(nc) as tc:
        _emit(nc, tc, dram, out_d, chain_d, taps, flags)
    return nc


def _emit(nc, tc, dram, out_d, chain_d, taps, flags):
    import contextlib
    pers_ctx = contextlib.ExitStack()
    pers = pers_ctx.enter_context(tc.tile_pool(name="pers", bufs=1))
    ctx = contextlib.ExitStack()
    with ctx:
        cst = ctx.enter_context(tc.tile_pool(name="cst", bufs=1))
        xm = ctx.enter_context(tc.tile_pool(name="xm", bufs=2))
        act = ctx.enter_context(tc.tile_pool(name="act", bufs=1))
        wts = ctx.enter_context(tc.tile_pool(name="wts", bufs=1))
        pp = ctx.enter_context(tc.tile_pool(name="pp", bufs=1, space="PSUM"))
        dpool = ctx.enter_context(tc.tile_pool(name="dpool", bufs=1, space="DRAM"))

        # ---- constants ----
        ones_col_f = cst.tile([128, 1], f32, name="ones_col_f")
        nc.vector.memset(ones_col_f, 1.0)
        ones_col = cst.tile([128, 1], f32r, name="ones_col")
        nc.scalar.copy(ones_col, ones_col_f)
        ones_row_f = cst.tile([1, 512], f32, name="ones_row_f")
        nc.vector.memset(ones_row_f, 1.0)
        ones_row = cst.tile([1, 512], f32r, name="ones_row")
        nc.scalar.copy(ones_row, ones_row_f)
        ones_col_bf = cst.tile([128, 1], bf16, name="ones_col_bf")
        nc.scalar.copy(ones_col_bf, ones_col_f)
        ident_bf = cst.tile([128, 128], bf16, name="ident_bf")
        make_identity(nc, ident_bf)
        ident_f = cst.tile([128, 128], f32, name="ident_f")
        make_identity(nc, ident_f)
        ident_r = cst.tile([128, 128], f32r, name="ident_r")
        nc.scalar.copy(ident_r, ident_f)
        eps_sb = cst.tile([1, 1], f32, name="eps_sb")
        nc.vector.memset(eps_sb, EPS)
        esel_sb = cst.tile([E, 128], f32r, name="esel_sb")
        nc.sync.dma_start(esel_sb, dram["esel"].ap())

        # copy engine alternator for plain psum->sbuf copies
        _eng = [0]

        def copy_ps(dst, src, bias=None):
            if bias is not None:
                nc.vector.tensor_scalar_add(dst, src, bias)
                return
            _eng[0] ^= 1
            if _eng[0]:
                nc.scalar.copy(dst, src)
            else:
                nc.vector.tensor_copy(dst, src)

        # ---- initial masters ----
        masters = {}
        x0t = xm.tile([128, DC, T_T], f32r, tag="x_t", name="x0_t")
        nc.sync.dma_start(x0t, dram["xt_t"].ap())
        masters["t"] = x0t
        x0c = xm.tile([128, DC, T_C], f32r, tag="x_c", name="x0_c")
        nc.sync.dma_start(x0c, dram["xt_c"].ap())
        masters["c"] = x0c

        # ---- mask bias rows (usually skipped: masks are all ones) ----
        mext = {}
        for tw in ("t", "c"):
            if flags[f"need_mask_{tw}"]:
                T = TOWER_T[tw]
                mrow = cst.tile([1, T], f32r, name=f"mrow_{tw}")
                nc.sync.dma_start(mrow, dram[f"mextrow_{tw}"].ap())
                m_ps = pp.tile([128, T], f32, tag="aux", bufs=2, name=_nm("mps"))
                nc.tensor.matmul(m_ps, ones_row[:, 0:128], mrow, start=True, stop=True)
                msb = cst.tile([128, T], f32, name=f"mext_{tw}")
                nc.scalar.copy(msb, m_ps)
                mext[tw] = msb

        # ---- layer norm emitter ----
        def emit_ln(y, lng_sb, lnb_sb, T, tw, dump=None):
            su_ps = pp.tile([1, T], f32, tag="aux", bufs=2, name=_nm("su"))
            sq_ps = pp.tile([1, T], f32, tag="aux", bufs=2, name=_nm("sq"))
            ysq = act.tile([128, DC, T], bf16, tag="sq", bufs=1, name=_nm("ysq"))
            nc.scalar.activation(ysq, y, AF.Square)
            for c in range(DC):
                nc.tensor.matmul(su_ps, ones_col, y[:, c, :],
                                 start=(c == 0), stop=(c == DC - 1))
            for c in range(DC):
                nc.tensor.matmul(sq_ps, ones_col_bf, ysq[:, c, :],
                                 start=(c == 0), stop=(c == DC - 1))
            m_sb = act.tile([1, T], f32, tag="st1", bufs=4, name=_nm("m"))
            nc.vector.tensor_scalar_mul(m_sb, su_ps, 1.0 / D)
            msq = act.tile([1, T], f32, tag="st1", bufs=4, name=_nm("msq"))
            nc.vector.tensor_mul(msq, m_sb, m_sb)
            var = act.tile([1, T], f32, tag="st1", bufs=4, name=_nm("var"))
            nc.vector.scalar_tensor_tensor(var, sq_ps, 1.0 / D, msq,
                                           op0=ALU.mult, op1=ALU.subtract)
            sd = act.tile([1, T], f32, tag="st1", bufs=4, name=_nm("sd"))
            nc.scalar.activation(sd, var, AF.Sqrt, bias=eps_sb[0:1, 0:1])
            a_sb = act.tile([1, T], f32, tag="st1", bufs=4, name=_nm("a"))
            nc.vector.reciprocal(a_sb, sd)
            a_r = act.tile([1, T], f32r, tag="st1", bufs=4, name=_nm("ar"))
            nc.scalar.copy(a_r, a_sb)
            b_r = act.tile([1, T], f32r, tag="st1", bufs=4, name=_nm("br"))
            nc.vector.scalar_tensor_tensor(b_r, m_sb, -1.0, a_sb,
                                           op0=ALU.mult, op1=ALU.mult)
            x_new = xm.tile([128, DC, T], f32r, tag=f"x_{tw}", name=_nm("x"))
            for c in range(DC):
                ag_ps = pp.tile([128, T], f32, tag="aux", bufs=2, name=_nm("ag"))
                nc.tensor.matmul(ag_ps, lng_sb[0:1, ts(c)], a_r,
                                 start=True, stop=True)
                bg_ps = pp.tile([128, T], f32, tag="aux", bufs=2, name=_nm("bg"))
                nc.tensor.matmul(bg_ps, lng_sb[0:1, ts(c)], b_r,
                                 start=True, stop=False)
                nc.tensor.matmul(bg_ps, lnb_sb[0:1, ts(c)], ones_row[:, 0:T],
                                 start=False, stop=True)
                tmp = act.tile([128, T], f32r, tag="lntmp", bufs=2, name=_nm("lt"))
                nc.vector.tensor_mul(tmp, y[:, c, :], ag_ps)
                nc.vector.tensor_add(x_new[:, c, :], tmp, bg_ps)
            if dump is not None:
                nc.sync.dma_start(dump.ap(), x_new.bitcast(f32))
            return x_new

        # ---- per-instance emission ----
        for key, tw, layer in INSTANCES:
            T = TOWER_T[tw]
            L = TOWER_L[tw]
            nT = T // 128
            nq = L // 128
            other = "c" if tw == "t" else "t"
            Lo = TOWER_L[other]
            x = masters[tw]
            xo = masters[other]

            # weights + params in
            wq = wts.tile([128, DC, D], bf16, tag="wp", bufs=2, name=_nm("wq"))
            nc.sync.dma_start(wq, dram[f"wq_{key}"].ap())
            wk = wts.tile([128, DC, D], bf16, tag="wp", bufs=2, name=_nm("wk"))
            nc.sync.dma_start(wk, dram[f"wk_{key}"].ap())
            wv = wts.tile([128, DC, D], bf16, tag="wp", bufs=2, name=_nm("wv"))
            nc.sync.dma_start(wv, dram[f"wv_{key}"].ap())
            wo = wts.tile([128, DC, D], bf16, tag="wp", bufs=2, name=_nm("wo"))
            nc.sync.dma_start(wo, dram[f"wo_{key}"].ap())

            def ld_small(nm_, shape, dt=f32):
                t_ = wts.tile(list(shape), dt, tag="bias", bufs=10, name=_nm(nm_))
                nc.sync.dma_start(t_, dram[f"{nm_}_{key}"].ap())
                return t_

            bq_sb = ld_small("bq", (128, DC))
            bk_sb = ld_small("bk", (128, DC))
            bo_sb = ld_small("bo", (128, DC))
            b1_sb = wts.tile([128, ICN], f32, tag="bias", bufs=10, name=_nm("b1"))
            nc.sync.dma_start(b1_sb, dram[f"b1_{key}"].ap())
            b2_sb = ld_small("b2", (128, DC))
            gw_sb = wts.tile([128, DC, E], f32r, tag="bias", bufs=10, name=_nm("gw"))
            nc.sync.dma_start(gw_sb, dram[f"gw_{key}"].ap())
            lnw = {}
            for ln_i in (1, 2):
                for gg in ("lng", "lnb"):
                    t_ = wts.tile([1, D], f32r, tag="lngb", bufs=5,
                                  name=_nm(f"{gg}{ln_i}"))
                    nc.sync.dma_start(t_, dram[f"{gg}{ln_i}_{key}"].ap())
                    lnw[f"{gg}{ln_i}"] = t_

            has_bq = flags[f"has_bq_{key}"]
            has_bk = flags[f"has_bk_{key}"]
            has_bo = flags[f"has_bo_{key}"]

            # shadow of x for bf16 matmuls
            xs = act.tile([128, DC, T], bf16, tag="shadow", bufs=2, name=_nm("xs"))
            nc.scalar.copy(xs, x)

            # --- Q, K projections (feature layout) ---
            q_bf = act.tile([128, DC, T], bf16, tag="qk", bufs=2, name=_nm("q"))
            k_bf = act.tile([128, DC, T], bf16, tag="qk", bufs=2, name=_nm("k"))
            for dst, w_t, b_t, hasb in ((q_bf, wq, bq_sb, has_bq),
                                        (k_bf, wk, bk_sb, has_bk)):
                for oc in range(DC):
                    ps = pp.tile([128, T], f32, tag="acc", bufs=3, name=_nm("ps"))
                    for d in range(DC):
                        nc.tensor.matmul(ps, w_t[:, d, ts(oc)], xs[:, d, :],
                                         start=(d == 0), stop=(d == DC - 1))
                    copy_ps(dst[:, oc, :], ps,
                            b_t[:, oc:oc + 1] if hasb else None)

            # --- q-fix: replace token 1 of each batch with cross-CLS query ---
            cc_bf = act.tile([128, DC, 2], bf16, tag="ccb", bufs=2, name=_nm("cc"))
            nc.vector.tensor_copy(cc_bf, xo[:, :, 0:Lo + 1:Lo])
            for oc in range(DC):
                ps = pp.tile([128, 2], f32, tag="acc", bufs=3, name=_nm("qc"))
                for d in range(DC):
                    nc.tensor.matmul(ps, wq[:, d, ts(oc)], cc_bf[:, d, :],
                                     start=(d == 0), stop=(d == DC - 1))
                copy_ps(q_bf[:, oc, 1:T:L], ps,
                        bq_sb[:, oc:oc + 1] if has_bq else None)

            # --- V (token layout) ---
            v_tiles = []
            for tt in range(nT):
                v_t = act.tile([128, D], bf16, tag="v", bufs=4, name=_nm("v"))
                for half in range(2):
                    ps = pp.tile([128, 384], f32, tag="acc", bufs=3, name=_nm("vp"))
                    for d in range(DC):
                        nc.tensor.matmul(ps, xs[:, d, ts(tt)],
                                         wv[:, d, half * 384:(half + 1) * 384],
                                         start=(d == 0), stop=(d == DC - 1))
                    copy_ps(v_t[:, half * 384:(half + 1) * 384], ps)
                if flags[f"need_bv_{key}"]:
                    bvr = cst.tile([1, D], f32r, name=_nm("bvr"))
                    nc.sync.dma_start(bvr, dram[f"bvrow_{key}"].ap())
                    bv_ps = pp.tile([128, D], f32, tag="aux", bufs=2, name=_nm("bvp"))
                    # note: D=768 > 512 psum limit; two bcasts
                    for hh in range(2):
                        nc.tensor.matmul(bv_ps[:, hh * 384:(hh + 1) * 384],
                                         ones_row[:, 0:128],
                                         bvr[:, hh * 384:(hh + 1) * 384],
                                         start=True, stop=True)
                    nc.vector.tensor_add(v_t, v_t, bv_ps)
                v_tiles.append(v_t)

            # --- attention: scores -> softmax -> transpose -> PV ---
            ctxT = act.tile([128, DC, T], bf16, tag="ctxT", bufs=1, name=_nm("ctxT"))
            for b in range(B):
                for ch in range(DC):
                    ctx_ps = pp.tile([128, L], f32, tag="acc", bufs=3, name=_nm("ctx"))
                    for s in range(2):
                        hoff = s * 64
                        pTs = [act.tile([128, L], bf16, tag="pT", bufs=3,
                                        name=_nm("pT")) for _ in range(nq)]
                        for qlt in range(nq):
                            sc_ps = pp.tile([128, L], f32, tag="sc", bufs=2,
                                            name=_nm("sc"))
                            nc.tensor.matmul(
                                sc_ps,
                                q_bf[hoff:hoff + 64, ch,
                                     b * L + qlt * 128: b * L + qlt * 128 + 128],
                                k_bf[hoff:hoff + 64, ch, b * L:(b + 1) * L],
                                start=True, stop=True)
                            if tw in mext:
                                nc.vector.tensor_add(
                                    sc_ps, sc_ps, mext[tw][:, b * L:(b + 1) * L])
                            mx = act.tile([128, 1], f32, tag="st1", bufs=4,
                                          name=_nm("mx"))
                            nc.vector.reduce_max(mx, sc_ps, axis=mybir.AxisListType.X)
                            nm8 = act.tile([128, 1], f32, tag="st1", bufs=4,
                                           name=_nm("nm8"))
                            nc.vector.tensor_scalar_mul(nm8, mx, -0.125)
                            probs = act.tile([128, L], bf16, tag="probs", bufs=2,
                                             name=_nm("pr"))
                            rs = act.tile([128, 1], f32, tag="st1", bufs=4,
                                          name=_nm("rs"))
                            nc.scalar.activation(probs, sc_ps, AF.Exp,
                                                 bias=nm8, scale=0.125,
                                                 accum_out=rs)
                            rec = act.tile([128, 1], f32, tag="st1", bufs=4,
                                           name=_nm("rc"))
                            nc.vector.reciprocal(rec, rs)
                            nc.vector.tensor_scalar_mul(probs, probs, rec)
                            for klt in range(nq):
                                tr_ps = pp.tile([128, 128], bf16, tag="tr", bufs=1,
                                                name=_nm("tr"))
                                nc.tensor.transpose(
                                    tr_ps, probs[:, klt * 128:(klt + 1) * 128],
                                    ident_bf)
                                nc.vector.tensor_copy(
                                    pTs[klt][:, qlt * 128:(qlt + 1) * 128], tr_ps)
                        for klt in range(nq):
                            nc.tensor.matmul(
                                ctx_ps[hoff:hoff + 64, :],
                                v_tiles[b * nq + klt][:, (2 * ch + s) * 64:
                                                      (2 * ch + s + 1) * 64],
                                pTs[klt],
                                start=(klt == 0), stop=(klt == nq - 1))
                    nc.scalar.copy(ctxT[:, ch, b * L:(b + 1) * L], ctx_ps)

            # --- O projection + residual -> y ---
            y = act.tile([128, DC, T], f32r, tag="b32", bufs=2, name=_nm("y"))
            for oc in range(DC):
                ps = pp.tile([128, T], f32, tag="acc", bufs=3, name=_nm("op"))
                for d in range(DC):
                    nc.tensor.matmul(ps, wo[:, d, ts(oc)], ctxT[:, d, :],
                                     start=(d == 0), stop=(d == DC - 1))
                if has_bo:
                    nc.vector.scalar_tensor_tensor(y[:, oc, :], ps,
                                                   bo_sb[:, oc:oc + 1],
                                                   x[:, oc, :],
                                                   op0=ALU.add, op1=ALU.add)
                else:
                    nc.vector.tensor_add(y[:, oc, :], ps, x[:, oc, :])
            if flags["tap"]:
                nc.sync.dma_start(taps[key]["y"].ap(), y.bitcast(f32))

            # --- LN1 -> x2 ---
            x2 = emit_ln(y, lnw["lng1"], lnw["lnb1"], T, tw,
                         dump=taps[key]["x2"] if flags["tap"] else None)

            # --- gate: top-2 of 8 -> per-core weight row, broadcast ---
            w8T = act.tile([E, T], f32r, tag="w8", bufs=1, name=_nm("w8T"))
            for tt in range(nT):
                g_ps = pp.tile([128, E], f32, tag="acc", bufs=3, name=_nm("gp"))
                last = DC - 1
                for d in range(DC):
                    nc.tensor.matmul(g_ps, x2[:, d, ts(tt)], gw_sb[:, d, :],
                                     start=(d == 0),
                                     stop=(d == last and
                                           not flags[f"need_gb_{key}"]))
                if flags[f"need_gb_{key}"]:
                    gbr = cst.tile([1, E], f32r, name=_nm("gbr"))
                    nc.sync.dma_start(gbr, dram[f"gbrow_{key}"].ap())
                    nc.tensor.matmul(g_ps, ones_row[:, 0:128], gbr,
                                     start=False, stop=True)
                mg = act.tile([128, 1], f32, tag="st1", bufs=4, name=_nm("mg"))
                nc.vector.reduce_max(mg, g_ps, axis=mybir.AxisListType.X)
                nmg = act.tile([128, 1], f32, tag="st1", bufs=4, name=_nm("nmg"))
                nc.vector.tensor_scalar_mul(nmg, mg, -1.0)
                e_sb = act.tile([128, E], f32, tag="gate", bufs=8, name=_nm("e"))
                nc.scalar.activation(e_sb, g_ps, AF.Exp, bias=nmg)
                m1 = act.tile([128, E], f32, tag="gate", bufs=8, name=_nm("m1"))
                nc.vector.tensor_single_scalar(m1, e_sb, 1.0, op=ALU.is_ge)
                em = act.tile([128, E], f32, tag="gate", bufs=8, name=_nm("em"))
                nc.vector.tensor_sub(em, e_sb, m1)
                e2 = act.tile([128, 1], f32, tag="st1", bufs=4, name=_nm("e2"))
                nc.vector.reduce_max(e2, em, axis=mybir.AxisListType.X)
                d1 = act.tile([128, 1], f32, tag="st1", bufs=4, name=_nm("d1"))
                nc.vector.tensor_scalar_add(d1, e2, 1.0)
                rcd = act.tile([128, 1], f32, tag="st1", bufs=4, name=_nm("rcd"))
                nc.vector.reciprocal(rcd, d1)
                tm = act.tile([128, E], f32, tag="gate", bufs=8, name=_nm("tm"))
                nc.vector.tensor_single_scalar(tm, e_sb, e2, op=ALU.is_ge)
                wt = act.tile([128, E], f32, tag="gate", bufs=8, name=_nm("wt"))
                nc.vector.tensor_mul(wt, e_sb, tm)
                wr = act.tile([128, E], f32r, tag="gate", bufs=8, name=_nm("wr"))
                nc.vector.tensor_scalar_mul(wr, wt, rcd)
                w8_ps = pp.tile([E, 128], f32r, tag="tr", bufs=1, name=_nm("w8p"))
                nc.tensor.transpose(w8_ps, wr, ident_r)
                nc.vector.tensor_copy(w8T[:, ts(tt)], w8_ps)
            if flags["tap"]:
                nc.sync.dma_start(taps[key]["w8"].ap(), w8T.bitcast(f32))
            wsel_ps = pp.tile([128, T], f32, tag="aux", bufs=2, name=_nm("wsp"))
            nc.tensor.matmul(wsel_ps, esel_sb, w8T, start=True, stop=True)
            wsel = act.tile([128, T], f32, tag="wsel", bufs=1, name=_nm("wsel"))
            nc.scalar.copy(wsel, wsel_ps)

            # --- MoE FFN (this core's expert, dense over all tokens) ---
            xs2 = act.tile([128, DC, T], bf16, tag="shadow", bufs=2, name=_nm("xs2"))
            nc.scalar.copy(xs2, x2)
            h_tiles = []
            for ic in range(ICN):
                w1t = wts.tile([128, DC * 128], bf16, tag="w1", bufs=4,
                               name=_nm("w1t"))
                nc.sync.dma_start(w1t, dram[f"w1_{key}"].ap()[ic])
                ps = pp.tile([128, T], f32, tag="acc", bufs=3, name=_nm("hp"))
                for d in range(DC):
                    nc.tensor.matmul(ps, w1t[:, ts(d)], xs2[:, d, :],
                                     start=(d == 0), stop=(d == DC - 1))
                h_bf = act.tile([128, T], bf16, tag="h", bufs=26, name=_nm("h"))
                nc.scalar.activation(h_bf, ps, AF.Gelu,
                                     bias=b1_sb[:, ic:ic + 1])
                h_tiles.append(h_bf)

            yw = act.tile([128, DC, T], f32, tag="b32", bufs=2, name=_nm("yw"))
            for oc in range(DC):
                ps = pp.tile([128, T], f32, tag="acc", bufs=3, name=_nm("yp"))
                for ic in range(ICN):
                    w2t = wts.tile([128, 128], bf16, tag="w2", bufs=6,
                                   name=_nm("w2t"))
                    nc.sync.dma_start(w2t, dram[f"w2_{key}"].ap()[oc, ic])
                    nc.tensor.matmul(ps, w2t, h_tiles[ic],
                                     start=(ic == 0), stop=(ic == ICN - 1))
                nc.vector.scalar_tensor_tensor(yw[:, oc, :], ps,
                                               b2_sb[:, oc:oc + 1], wsel,
                                               op0=ALU.add, op1=ALU.mult)

            # --- AllReduce combine across the 8 experts ---
            ccin = dpool.tile([128, DC * T], f32, tag=f"ccin_{tw}", bufs=2,
                              name=_nm("ccin"))
            nc.sync.dma_start(ccin, yw)
            ccout = dpool.tile([128, DC * T], f32, tag=f"ccout_{tw}", bufs=2,
                               addr_space="Shared", name=_nm("ccout"))
            nc.gpsimd.collective_compute(
                "AllReduce", ALU.add,
                replica_groups=[list(range(N_CORES))],
                ins=[ccin[:, :]],
                outs=[ccout[:, :]],
            )
            msum = act.tile([128, DC, T], f32, tag="b32", bufs=2, name=_nm("ms"))
            nc.sync.dma_start(msum, ccout[:, :])
            if flags["tap"]:
                nc.sync.dma_start(taps[key]["ms"].ap(), msum)

            y2 = act.tile([128, DC, T], f32r, tag="b32", bufs=2, name=_nm("y2"))
            nc.vector.tensor_add(y2, x2, msum)

            # --- LN2 -> new master ---
            masters[tw] = emit_ln(y2, lnw["lng2"], lnw["lnb2"], T, tw,
                                  dump=taps[key]["x"] if flags["tap"] else None)

        # CLS feature columns survive the main pools for the classifier
        ft = {}
        for tw2 in ("t", "c"):
            L2 = TOWER_L[tw2]
            ftt = pers.tile([128, DC, 2], f32r, name=f"ft_{tw2}")
            nc.vector.tensor_copy(ftt, masters[tw2][:, :, 0:L2 + 1:L2])
            ft[tw2] = ftt

    # ---- classifier ----
    with (
        tc.tile_pool(name="clf", bufs=1) as clf,
        tc.tile_pool(name="clfp", bufs=1, space="PSUM") as clfp,
    ):
        b1c_sb = clf.tile([128, DC], f32, tag="cb", bufs=4, name="b1c_sb")
        nc.sync.dma_start(b1c_sb, dram["b1c"].ap())
        w2c_sb = clf.tile([128, DC, E], f32r, tag="cb", bufs=4, name="w2c_sb")
        nc.sync.dma_start(w2c_sb, dram["w2c"].ap())
        ef_sb = clf.tile([4, 2], f32r, tag="cb", bufs=4, name="ef_sb")
        nc.sync.dma_start(ef_sb, dram["efT"].ap())
        w1cb_sb = clf.tile([4, D], f32r, tag="cb", bufs=4, name="w1cb_sb")
        nc.sync.dma_start(w1cb_sb, dram["w1cb"].ap())

        h1 = clf.tile([128, DC, 2], f32r, tag="h1", name="h1")
        w1ca_tiles = []
        for kc in range(12):
            w1t = clf.tile([128, D], f32r, tag="w1c", bufs=13, name=_nm("w1ct"))
            nc.sync.dma_start(w1t, dram["w1ca"].ap()[kc])
            w1ca_tiles.append(w1t)
        for oc in range(DC):
            ps = clfp.tile([128, 2], f32, tag="cacc", bufs=2, name=_nm("cp"))
            for kc in range(13):
                if kc < 6:
                    rhs = ft["t"][:, kc, :]
                    lhsT = w1ca_tiles[kc][:, ts(oc)]
                elif kc < 12:
                    rhs = ft["c"][:, kc - 6, :]
                    lhsT = w1ca_tiles[kc][:, ts(oc)]
                else:
                    rhs = ef_sb
                    lhsT = w1cb_sb[:, ts(oc)]
                nc.tensor.matmul(ps, lhsT, rhs, start=(kc == 0), stop=(kc == 12))
            nc.scalar.activation(h1[:, oc, :], ps, AF.Relu,
                                 bias=b1c_sb[:, oc:oc + 1])
        out_ps = clfp.tile([2, 8], f32, tag="cacc", bufs=2, name="out_ps")
        for oc in range(DC):
            nc.tensor.matmul(out_ps, h1[:, oc, :], w2c_sb[:, oc, :],
                             start=(oc == 0),
                             stop=(oc == DC - 1 and not flags["need_b2c"]))
        if flags["need_b2c"]:
            b2c_sb = clf.tile([1, 8], f32r, tag="cb", bufs=4, name="b2c_sb")
            nc.sync.dma_start(b2c_sb, dram["b2crow"].ap())
            ones2 = clf.tile([1, 2], f32, tag="cb", bufs=4, name="ones2")
            nc.vector.memset(ones2, 1.0)
            ones2r = clf.tile([1, 2], f32r, tag="cb", bufs=4, name="ones2r")
            nc.scalar.copy(ones2r, ones2)
            nc.tensor.matmul(out_ps, ones2r, b2c_sb, start=False, stop=True)
        out_sb = clf.tile([2, 8], f32, tag="cb", bufs=4, name="out_sb")
        nc.scalar.copy(out_sb, out_ps)
        nc.sync.dma_start(out_d.ap(), out_sb)

        # chain passthrough for timing loops
        ch_sb = clf.tile([1, 1], f32, tag="cb", bufs=4, name="ch_sb")
        nc.sync.dma_start(ch_sb, dram["chain"].ap())
        ch_out = clf.tile([1, 1], f32, tag="cb", bufs=4, name="ch_out")
        nc.vector.tensor_add(ch_out, ch_sb, out_sb[0:1, 0:1])
        nc.sync.dma_start(chain_d.ap(), ch_out)
    pers_ctx.close()


# ============================================================================
# host side
# ============================================================================

def _f32(a):
    return np.ascontiguousarray(np.asarray(a, dtype=np.float32))


def _feat_chunks(m):
    """[D, N] -> [128, DC, N] (feature-partition layout)."""
    Dd, N = m.shape
    return np.ascontiguousarray(m.reshape(DC, 128, N).transpose(1, 0, 2))


def _col_per_chunk(v, n):
    """[n*128] -> [128, n] (per-partition scalar columns)."""
    return np.ascontiguousarray(v.reshape(n, 128).T)


def make_host_inputs(text_ids, text_mask, code_ids, code_type_ids, code_mask,
                     explicit_feat, t_params, c_params, clf_params):
    """Returns (flags, shared_inputs, per_core_only) ready for the device."""
    tp = {k: np.asarray(v) for k, v in t_params.items()}
    cp = {k: np.asarray(v) for k, v in c_params.items()}
    fp = {k: np.asarray(v) for k, v in clf_params.items()}
    text_ids = np.asarray(text_ids)
    code_ids = np.asarray(code_ids)
    code_type_ids = np.asarray(code_type_ids)
    text_mask = np.asarray(text_mask)
    code_mask = np.asarray(code_mask)
    ef = _f32(explicit_feat)

    # embeddings on host (gather + add), fp32
    t_hid = _f32(tp["word"])[text_ids] + _f32(tp["pos"])[None, :LT, :]
    t_hid[:, :2, :] = _f32(tp["word"])[0]
    c_hid = (_f32(cp["word"])[code_ids] + _f32(cp["pos"])[None, :LC, :]
             + _f32(cp["type"])[code_type_ids])
    c_hid[:, :2, :] = _f32(cp["word"])[0]

    flags = {"tap": False}
    flags["need_mask_t"] = not np.all(text_mask == 1)
    flags["need_mask_c"] = not np.all(code_mask == 1)
    shared = {
        "xt_t": _feat_chunks(t_hid.reshape(T_T, D).T.astype(np.float32)),
        "xt_c": _feat_chunks(c_hid.reshape(T_C, D).T.astype(np.float32)),
        "efT": _f32(ef.T),
    }
    if flags["need_mask_t"]:
        shared["mextrow_t"] = _f32(
            ((1.0 - text_mask.astype(np.float32)) * NEG).reshape(1, T_T))
    if flags["need_mask_c"]:
        shared["mextrow_c"] = _f32(
            ((1.0 - code_mask.astype(np.float32)) * NEG).reshape(1, T_C))

    for key, tw, i in INSTANCES:
        p = tp if tw == "t" else cp
        for mm, nm_ in (("Wq", "wq"), ("Wk", "wk"), ("Wv", "wv"), ("Wo", "wo")):
            w = _f32(p[mm][i])
            shared[f"{nm_}_{key}"] = np.ascontiguousarray(
                w.reshape(DC, 128, D).transpose(1, 0, 2).astype(BF16))
        for bb, nm_ in (("bq", "bq"), ("bk", "bk"), ("bo", "bo")):
            v = _f32(p[bb][i])
            flags[f"has_{nm_}_{key}"] = bool(np.any(v != 0))
            shared[f"{nm_}_{key}"] = _col_per_chunk(v, DC)
        bv = _f32(p["bv"][i])
        flags[f"need_bv_{key}"] = bool(np.any(bv != 0))
        if flags[f"need_bv_{key}"]:
            shared[f"bvrow_{key}"] = _f32(bv.reshape(1, D))
        gw = _f32(p["gW"][i])
        shared[f"gw_{key}"] = np.ascontiguousarray(
            gw.reshape(DC, 128, E).transpose(1, 0, 2))
        gb = _f32(p["gb"][i])
        flags[f"need_gb_{key}"] = bool(np.any(gb != 0))
        if flags[f"need_gb_{key}"]:
            shared[f"gbrow_{key}"] = _f32(gb.reshape(1, E))
        shared[f"lng1_{key}"] = _f32(p["ln1g"][i]).reshape(1, D)
        shared[f"lnb1_{key}"] = _f32(p["ln1b"][i]).reshape(1, D)
        shared[f"lng2_{key}"] = _f32(p["ln2g"][i]).reshape(1, D)
        shared[f"lnb2_{key}"] = _f32(p["ln2b"][i]).reshape(1, D)

    w1c = _f32(fp["W1"])
    shared["w1ca"] = np.ascontiguousarray(w1c[:1536].reshape(12, 128, D))
    shared["w1cb"] = _f32(w1c[1536:])
    shared["b1c"] = _col_per_chunk(_f32(fp["b1"]), DC)
    w2c = _f32(fp["W2"])
    shared["w2c"] = np.ascontiguousarray(
        w2c.reshape(DC, 128, 8).transpose(1, 0, 2))
    b2c = _f32(fp["b2"])
    flags["need_b2c"] = bool(np.any(b2c != 0))
    if flags["need_b2c"]:
        shared["b2crow"] = _f32(b2c.reshape(1, 8))
    shared["chain"] = np.zeros((1, 1), np.float32)

    # per-core expert weights
    per_core = []
    for e in range(N_CORES):
        pc = {}
        sel = np.zeros((E, 128), np.float32)
        sel[e, :] = 1.0
        pc["esel"] = sel
        for key, tw, i in INSTANCES:
            p = tp if tw == "t" else cp
            w1 = _f32(p["W1"][i][e])     # [768, 3072]
            pc[f"w1_{key}"] = np.ascontiguousarray(
                w1.reshape(DC, 128, ICN, 128).transpose(2, 1, 0, 3).astype(BF16))
            pc[f"b1_{key}"] = _col_per_chunk(_f32(p["b1"][i][e]), ICN)
            w2 = _f32(p["W2"][i][e])     # [3072, 768]
            pc[f"w2_{key}"] = np.ascontiguousarray(
                w2.reshape(ICN, 128, DC, 128).transpose(2, 0, 1, 3).astype(BF16))
            pc[f"b2_{key}"] = _col_per_chunk(_f32(p["b2"][i][e]), DC)
        per_core.append(pc)

    return flags, shared, per_core


# ---- cached PJRT runner ----
_RUNNER_CACHE = {}


class _Runner:
    def __init__(self, nc, n_cores):
        import jax
        from jax.sharding import Mesh, PartitionSpec
        from jax.experimental.shard_map import shard_map
        from concourse import bass2jax

        bass2jax.install_neuronx_cc_hook()
        self.jax = jax
        self.n_cores = n_cores

        partition_name = (nc.partition_id_tensor.name
                          if nc.partition_id_tensor else None)
        in_names, out_names, out_avals, zero_outs = [], [], [], []
        for alloc in nc.m.functions[0].allocations:
            if not isinstance(alloc, mybir.MemoryLocationSet):
                continue
            name = alloc.memorylocations[0].name
            if alloc.kind == "ExternalInput":
                if name != partition_name:
                    in_names.append(name)
            elif alloc.kind == "ExternalOutput":
                out_names.append(name)
                shape = tuple(alloc.tensor_shape)
                dtype = mybir.dt.np(alloc.dtype)
                out_avals.append(jax.core.ShapedArray(shape, dtype))
                zero_outs.append(np.zeros(shape, dtype))
        self.in_names = in_names
        self.out_names = out_names
        self.out_avals = out_avals
        self.zero_outs = zero_outs
        n_params = len(in_names)
        self.n_params = n_params
        n_outs = len(out_avals)

        all_in_names = list(in_names) + list(out_names)
        if partition_name is not None:
            all_in_names.append(partition_name)

        def _body(*args):
            operands = list(args)
            if partition_name is not None:
                operands.append(bass2jax.partition_id_tensor())
            outs = bass2jax._bass_exec_p.bind(
                *operands,
                out_avals=tuple(out_avals),
                in_names=tuple(all_in_names),
                out_names=tuple(out_names),
                lowering_input_output_aliases=(),
                sim_require_finite=True,
                sim_require_nnan=True,
                nc=nc,
            )
            return tuple(outs)

        devices = jax.devices()[:n_cores]
        mesh = Mesh(np.asarray(devices), ("core",))
        in_specs = (PartitionSpec("core"),) * (n_params + n_outs)
        out_specs = (PartitionSpec("core"),) * n_outs
        self.fn = jax.jit(shard_map(
            _body, mesh=mesh, in_specs=in_specs, out_specs=out_specs,
            check_rep=False), keep_unused=True)
        self.mesh = mesh
        self._dev_args = None

    def _device_args(self, in_maps):
        from jax.sharding import NamedSharding, PartitionSpec
        n = self.n_cores
        sh = NamedSharding(self.mesh, PartitionSpec("core"))
        concat_in = [
            np.concatenate([np.asarray(in_maps[c][k]) for c in range(n)], axis=0)
            for k in self.in_names
        ]
        concat_zero = [
            np.zeros((n * z.shape[0], *z.shape[1:]), z.dtype)
            for z in self.zero_outs
        ]
        return [self.jax.device_put(a, sh) for a in concat_in + concat_zero]

    def __call__(self, in_maps):
        n = self.n_cores
        if self._dev_args is None:
            self._dev_args = self._device_args(in_maps)
        out_arrs = self.fn(*self._dev_args)
        out_arrs = [np.asarray(o) for o in out_arrs]
        return [
            {
                name: out_arrs[i].reshape(n, *self.out_avals[i].shape)[c]
                for i, name in enumerate(self.out_names)
            }
            for c in range(n)
        ]

    def time_exec(self, in_maps, reps=12):
        import time as _t
        if self._dev_args is None:
            self._dev_args = self._device_args(in_maps)
        for _ in range(2):
            o = self.fn(*self._dev_args)
            self.jax.block_until_ready(o)
        times = []
        for _ in range(reps):
            t0 = _t.perf_counter()
            o = self.fn(*self._dev_args)
            self.jax.block_until_ready(o)
            times.append(_t.perf_counter() - t0)
        return times


def get_runner(flags):
    key = tuple(sorted(flags.items()))
    if key not in _RUNNER_CACHE:
        nc = build_nc(flags)
        nc.finalize()
        _RUNNER_CACHE[key] = _Runner(nc, N_CORES)
    return _RUNNER_CACHE[key]


def _make_in_maps(flags, shared, per_core):
    return [dict(shared, **pc) for pc in per_core]


def kernel(**inputs) -> np.ndarray:
    flags, shared, per_core = make_host_inputs(**inputs)
    runner = get_runner(flags)
    res = runner(_make_in_maps(flags, shared, per_core))
    return np.ascontiguousarray(res[0]["out"])


if __name__ == "__main__":
    import reference
    inp = reference.setup_inputs()
    exp = np.asarray(reference.reference(**inp))
    got = kernel(**{k: v for k, v in inp.items()})
    rel = np.linalg.norm(got - exp) / np.linalg.norm(exp)
    print("out:", got)
    print("exp:", exp)
    print("Relative error:", rel)
